# revision 13
# baseline (speedup 1.0000x reference)
"""Trainium2 Bass kernel for DecoderWithAttention (show-attend-tell style).

Strategy: data-parallel over batch across 8 cores (B=128 -> 16 per core).
Per core, everything runs on one NeuronCore:
  phase A: load encoder shard (bf16, SBUF-resident), hoist att1 = enc @ W_enc_att.T,
           E_proj = emb[captions] @ W_ih[:, :EMB].T + b_ih + b_hh, h0/c0 from mean(enc).
  phase B: 20 sequential decode steps (attention scores via relu + PE matvec,
           softmax, context via block-diagonal alpha matmul against resident encoder,
           gated context, LSTM cell). Weights for the recurrent matmuls stream from HBM.
  phase C: predictions = H @ W_fc.T as one batched matmul, PSUM -> DRAM directly.
"""

import numpy as np
import ml_dtypes
from contextlib import ExitStack

import concourse.bass as bass
import concourse.mybir as mybir
import concourse.tile as tile
from concourse import bacc
from concourse.bass_utils import run_bass_kernel_spmd
from concourse.masks import make_identity

F32 = mybir.dt.float32
BF16 = mybir.dt.bfloat16
I32 = mybir.dt.int32
AF = mybir.ActivationFunctionType
OP = mybir.AluOpType

# model dims
NCORES = 8
B = 128
BL = B // NCORES          # 16 batch per core
R = 196                   # regions
E = 2048                  # encoder dim
A = 512                   # attention dim
D = 512                   # decoder dim
G = 4 * D                 # lstm gates dim (2048)
EMB = 512
V = 10000
T = 21 - 1                # decode steps
P = 128

BR = BL * R               # 3136
NCH = (BR + P - 1) // P   # 25 chunks of flat (b, r)
BRP = NCH * P             # 3200 padded rows
ACH = A // P              # 4
ECH = E // P              # 16
GCH = G // P              # 16
DCH = D // P              # 4
MCH = EMB // P            # 4
VS = 512                  # vocab slice for phase C
VP = ((V + VS - 1) // VS) * VS   # 10240
NVS = VP // VS            # 20
ANS = 224                 # att1 (b,r) slice width
NNS = BR // ANS           # 7

_CACHE = {}


def _bf(x):
    return np.asarray(x, dtype=ml_dtypes.bfloat16)


def _tiles(wt, kch, mch):
    """Pre-tile a [K, M] matrix into [mch, kch, 128, 128] lhsT tiles."""
    Kd, Md = wt.shape
    assert Kd == kch * P and Md == mch * P
    return np.ascontiguousarray(wt.reshape(kch, P, mch, P).transpose(2, 0, 1, 3))


def build_nc(t_steps=T, need_bfc=False):
    nt = BL * t_steps
    ntp = ((nt + P - 1) // P) * P
    nc = bacc.Bacc(None, target_bir_lowering=False)

    # ---- DRAM I/O ----
    enc_d = nc.dram_tensor("enc", [BRP, E], BF16, kind="ExternalInput")
    encT_d = nc.dram_tensor("encT", [E, BR], BF16, kind="ExternalInput")
    cap_d = nc.dram_tensor("cap", [ntp], I32, kind="ExternalInput")
    emb_d = nc.dram_tensor("emb", [V, EMB], F32, kind="ExternalInput")
    wE_d = nc.dram_tensor("wE", [ACH, ECH, P, P], BF16, kind="ExternalInput")
    bEnc_d = nc.dram_tensor("bEnc", [P, ACH], F32, kind="ExternalInput")
    bDec_d = nc.dram_tensor("bDec", [P, ACH], F32, kind="ExternalInput")
    wHcat_d = nc.dram_tensor("wHcat", [ACH + ECH + GCH, DCH, P, P], BF16,
                             kind="ExternalInput")
    wIc_d = nc.dram_tensor("wIc", [GCH, ECH, P, P], BF16, kind="ExternalInput")
    wIe_d = nc.dram_tensor("wIe", [GCH, MCH, P, P], BF16, kind="ExternalInput")
    biasG_d = nc.dram_tensor("biasG", [P, GCH], F32, kind="ExternalInput")
    bFbeta_d = nc.dram_tensor("bFbeta", [P, ECH], F32, kind="ExternalInput")
    wFull_d = nc.dram_tensor("wFull", [P, ACH], BF16, kind="ExternalInput")
    wInitH_d = nc.dram_tensor("wInitH", [DCH, ECH, P, P], BF16, kind="ExternalInput")
    wInitC_d = nc.dram_tensor("wInitC", [DCH, ECH, P, P], BF16, kind="ExternalInput")
    bInitH_d = nc.dram_tensor("bInitH", [P, DCH], F32, kind="ExternalInput")
    bInitC_d = nc.dram_tensor("bInitC", [P, DCH], F32, kind="ExternalInput")
    maskA_d = nc.dram_tensor("maskA", [P, 2 * BL], F32, kind="ExternalInput")
    onesBlk_d = nc.dram_tensor("onesBlk", [BRP, BL], BF16, kind="ExternalInput")
    wFc_d = nc.dram_tensor("wFc", [NVS, DCH, P, VS], BF16, kind="ExternalInput")
    bFc_d = nc.dram_tensor("bFc", [1, VP], BF16, kind="ExternalInput")

    preds_d = nc.dram_tensor("preds", [BL * t_steps, V], F32, kind="ExternalOutput")
    alphas_d = nc.dram_tensor("alphas", [BL, t_steps, R], F32, kind="ExternalOutput")

    with tile.TileContext(nc) as tc, ExitStack() as ctx:
        const = ctx.enter_context(tc.tile_pool(name="const", bufs=1))
        work = ctx.enter_context(tc.tile_pool(name="work", bufs=2))
        wstream = ctx.enter_context(tc.tile_pool(name="wstream", bufs=3))
        bigs = ctx.enter_context(tc.tile_pool(name="bigs", bufs=2))
        psA = ctx.enter_context(tc.tile_pool(name="psA", bufs=2, space="PSUM"))
        psSmall = ctx.enter_context(tc.tile_pool(name="psSmall", bufs=2, space="PSUM"))
        psUnit = ctx.enter_context(tc.tile_pool(name="psUnit", bufs=3, space="PSUM"))
        psScore = ctx.enter_context(tc.tile_pool(name="psScore", bufs=1, space="PSUM"))

        # ---- resident tiles ----
        enc_sb = const.tile([P, NCH, E], BF16, tag="enc_sb")
        att1 = const.tile([P, ACH, BR], BF16, tag="att1")
        eproj = const.tile([P, GCH, t_steps, BL], BF16, tag="eproj")
        Hbf = const.tile([P, DCH, BL, t_steps + 1], BF16, tag="Hbf")
        A_sb = const.tile([P, NCH, BL], BF16, tag="A_sb")
        ident_bf = const.tile([P, P], BF16, tag="ident_bf")
        ident_f = const.tile([P, P], F32, tag="ident_f")
        ones1f = const.tile([P, 1], F32, tag="ones1f")
        onesRowF = const.tile([1, P], F32, tag="onesRowF")
        rec2 = const.tile([1, 2 * BL], F32, tag="rec2")
        onesRow = const.tile([1, P], BF16, tag="onesRow")
        bEnc = const.tile([P, ACH], F32, tag="bEnc")
        bDec = const.tile([P, ACH], F32, tag="bDec")
        biasG = const.tile([P, GCH], F32, tag="biasG")
        bFbeta = const.tile([P, ECH], F32, tag="bFbeta")
        wFull = const.tile([P, ACH], BF16, tag="wFull")
        bInitH = const.tile([P, DCH], F32, tag="bInitH")
        bInitC = const.tile([P, DCH], F32, tag="bInitC")
        maskA = const.tile([P, 2 * BL], F32, tag="maskA")
        bFc = const.tile([1, VP], BF16, tag="bFc")
        cap_sb = const.tile([P, ntp // P], I32, tag="cap_sb")
        c_st = [const.tile([P, DCH, BL], F32, tag=f"c_st{i}", name=f"c_st{i}")
                for i in range(2)]
        embT = const.tile([P, MCH, ntp], BF16, tag="embT")
        meanT = const.tile([P, ECH, BL], BF16, tag="meanT")
        gsum = const.tile([P, GCH, BL], F32, tag="gsum")
        g_sb = const.tile([P, ECH, BL], F32, tag="g_sb")
        gcT = const.tile([P, ECH, BL], BF16, tag="gcT")
        act4 = const.tile([P, GCH, BL], F32, tag="act4")
        sc_cl = const.tile([P, 2 * BL], F32, tag="sc_cl")
        expm = const.tile([P, 2 * BL], F32, tag="expm")
        alpha_f = const.tile([P, 2 * BL], F32, tag="alpha_f")
        alpha_bf = const.tile([P, 2 * BL], BF16, tag="alpha_bf")
        den = const.tile([1, BL], F32, tag="den")
        sums_sb = const.tile([1, 2 * BL], F32, tag="sums_sb")
        rec = const.tile([1, BL], F32, tag="rec")
        tanh_c = const.tile([P, DCH, BL], F32, tag="tanh_c")
        itg = const.tile([P, DCH, BL], F32, tag="itg")

        scoreT = psScore.tile([P, 2 * BL, ACH], F32, tag="scoreT")

        make_identity(nc, ident_bf[:])
        make_identity(nc, ident_f[:])
        nc.vector.memset(ones1f[:], 1.0)
        nc.vector.memset(onesRowF[:], 1.0)
        nc.vector.memset(onesRow[:], 1.0)
        nc.vector.memset(scoreT[:], 0.0)

        # ---- small loads ----
        nc.sync.dma_start(bEnc[:], bEnc_d[:, :])
        nc.sync.dma_start(bDec[:], bDec_d[:, :])
        nc.sync.dma_start(biasG[:], biasG_d[:, :])
        nc.sync.dma_start(bFbeta[:], bFbeta_d[:, :])
        nc.sync.dma_start(wFull[:], wFull_d[:, :])
        nc.sync.dma_start(bInitH[:], bInitH_d[:, :])
        nc.sync.dma_start(bInitC[:], bInitC_d[:, :])
        nc.sync.dma_start(maskA[:], maskA_d[:, :])
        nc.sync.dma_start(bFc[:], bFc_d[:, :])
        nc.sync.dma_start(cap_sb[:], cap_d.ap().rearrange("(c p) -> p c", p=P))
        # big loads: encoder resident + ones-block into A_sb (reused for mean)
        nc.sync.dma_start(enc_sb[:], enc_d.ap().rearrange("(c p) e -> p c e", p=P))
        nc.sync.dma_start(A_sb[:], onesBlk_d.ap().rearrange("(c p) m -> p c m", p=P))

        # ---- phase A: embeddings gather + E_proj ----
        emb_g = bigs.tile([P, ntp // P, EMB], F32, tag="bigscratch")
        for j in range(ntp // P):
            nc.gpsimd.indirect_dma_start(
                out=emb_g[:, j, :], out_offset=None, in_=emb_d[:, :],
                in_offset=bass.IndirectOffsetOnAxis(ap=cap_sb[:, j:j + 1], axis=0),
            )
        for j in range(ntp // P):
            for ec in range(MCH):
                tp = psSmall.tile([P, P], F32, tag="ps_small")
                nc.tensor.transpose(tp[:], emb_g[:, j, ec * P:(ec + 1) * P],
                                    ident_f[:])
                nc.vector.tensor_copy(embT[:, ec, j * P:(j + 1) * P], tp[:])
        for gc in range(GCH):
            wie_t = wstream.tile([P, ECH, P], BF16, tag="wst")
            nc.sync.dma_start(wie_t[:, :MCH, :],
                              wIe_d.ap()[gc].rearrange("c p o -> p c o"))
            ep_ps = psA.tile([P, VS], F32, tag="bankA")
            for ec in range(MCH):
                nc.tensor.matmul(ep_ps[:, :nt], wie_t[:, ec, :], embT[:, ec, :nt],
                                 start=(ec == 0), stop=(ec == MCH - 1))
            # psum cols are (b, t) b-major; eproj wants [gch, t, b]
            nc.scalar.activation(
                eproj[:, gc, :, :],
                ep_ps[:, :nt].rearrange("p (b t) -> p t b", b=BL),
                AF.Identity, bias=biasG[:, gc:gc + 1])

        # ---- phase A: mean encoder -> h0, c0 ----
        for q in range(4):
            mn_ps = psA.tile([BL, VS], F32, tag="bankA")
            for c in range(NCH):
                nc.tensor.matmul(mn_ps[:], A_sb[:, c, :],
                                 enc_sb[:, c, q * VS:(q + 1) * VS],
                                 start=(c == 0), stop=(c == NCH - 1))
            mean_bf = work.tile([BL, VS], BF16, tag="ctx_bf")
            nc.scalar.copy(mean_bf[:], mn_ps[:])
            for e4 in range(VS // P):
                ec = q * (VS // P) + e4
                tp = psSmall.tile([P, BL], BF16, tag="ps_small")
                nc.tensor.transpose(tp[:], mean_bf[:, e4 * P:(e4 + 1) * P],
                                    ident_bf[:BL, :BL])
                nc.vector.tensor_copy(meanT[:, ec, :], tp[:])
        for w_d, bias_t, kind in ((wInitH_d, bInitH, "h"), (wInitC_d, bInitC, "c")):
            for dc in range(DCH):
                wi_t = wstream.tile([P, ECH, P], BF16, tag="wst")
                nc.sync.dma_start(wi_t[:], w_d.ap()[dc].rearrange("c p o -> p c o"))
                hc_ps = psSmall.tile([P, BL], F32, tag="ps_small")
                for ec in range(ECH):
                    nc.tensor.matmul(hc_ps[:], wi_t[:, ec, :], meanT[:, ec, :],
                                     start=(ec == 0), stop=(ec == ECH - 1))
                if kind == "h":
                    nc.scalar.activation(Hbf[:, dc, :, 0], hc_ps[:], AF.Identity,
                                         bias=bias_t[:, dc:dc + 1])
                else:
                    nc.scalar.activation(c_st[0][:, dc, :], hc_ps[:], AF.Identity,
                                         bias=bias_t[:, dc:dc + 1])

        # ---- phase A: att1 = enc @ W_enc_att.T (+bias), stored transposed ----
        for ns in range(NNS):
            encT_sl = bigs.tile([P, ECH, ANS], BF16, tag="bigscratch")
            nc.sync.dma_start(
                encT_sl[:],
                encT_d.ap().rearrange("(c p) n -> p c n", p=P)
                [:, :, ns * ANS:(ns + 1) * ANS])
            for ac in range(ACH):
                wE_t = wstream.tile([P, ECH, P], BF16, tag="wst")
                nc.sync.dma_start(wE_t[:], wE_d.ap()[ac].rearrange("c p o -> p c o"))
                a1_ps = psA.tile([P, ANS], F32, tag="bankA")
                for ec in range(ECH):
                    nc.tensor.matmul(a1_ps[:], wE_t[:, ec, :], encT_sl[:, ec, :],
                                     start=(ec == 0), stop=(ec == ECH - 1))
                nc.scalar.activation(att1[:, ac, ns * ANS:(ns + 1) * ANS], a1_ps[:],
                                     AF.Identity, bias=bEnc[:, ac:ac + 1])

        # A_sb now becomes the alpha block-diagonal matrix
        nc.vector.memset(A_sb[:], 0.0)

        # scatter plan for alpha columns -> flat (b, r) rows of A_sb
        scat = []
        for b in range(BL):
            for rc in range(2):
                g0 = b * R + rc * P
                total = P if rc == 0 else R - P
                cnt = total
                while cnt > 0:
                    c0, p0 = divmod(g0, P)
                    n1 = min(cnt, P - p0)
                    s0 = total - cnt
                    scat.append((b, rc, s0, c0, p0, n1))
                    g0 += n1
                    cnt -= n1

        # ---- phase B: decode steps ----
        for t in range(t_steps):
            h_rhs = Hbf[:, :, :, t]

            # att2 = h @ W_dec_att.T (transposed layout [a, b])
            att2_ps = psSmall.tile([P, ACH, BL], F32, tag="ps_small")
            for oc in range(ACH):
                wh_t = wstream.tile([P, ECH, P], BF16, tag="wst")
                nc.sync.dma_start(wh_t[:, :DCH, :],
                                  wHcat_d.ap()[oc].rearrange("c p o -> p c o"))
                for dc in range(DCH):
                    nc.tensor.matmul(att2_ps[:, oc, :], wh_t[:, dc, :],
                                     h_rhs[:, dc, :],
                                     start=(dc == 0), stop=(dc == DCH - 1))
            att2_bf = work.tile([P, ACH, BL], BF16, tag="att2_bf")
            for ac in range(ACH):
                nc.scalar.activation(att2_bf[:, ac, :], att2_ps[:, ac, :],
                                     AF.Identity, bias=bDec[:, ac:ac + 1])

            # relu(att1 + att2) and score matvec, per a-chunk
            for ac in range(ACH):
                tmp = bigs.tile([P, BL, R], BF16, tag="bigscratch")
                nc.vector.tensor_tensor(
                    out=tmp[:],
                    in0=att1[:, ac, :].rearrange("p (b r) -> p b r", b=BL),
                    in1=att2_bf[:, ac, :, None].to_broadcast([P, BL, R]),
                    op=OP.add)
                nc.scalar.activation(tmp[:], tmp[:], AF.Relu)
                for b in range(BL):
                    for rc in range(2):
                        cnt = P if rc == 0 else R - P
                        nc.tensor.matmul(
                            scoreT[:cnt, 2 * b + rc, ac:ac + 1],
                            tmp[:, b, rc * P:rc * P + cnt],
                            wFull[:, ac:ac + 1],
                            start=True, stop=True)

            # softmax over r (columns hold (b, rc) pairs)
            nc.vector.tensor_reduce(out=sc_cl[:, :, None], in_=scoreT[:],
                                    axis=mybir.AxisListType.X, op=OP.add)
            nc.vector.tensor_scalar(out=expm[:], in0=sc_cl[:], scalar1=30.0,
                                    scalar2=-30.0, op0=OP.min, op1=OP.max)
            nc.scalar.activation(sc_cl[:], expm[:], AF.Exp)
            nc.vector.tensor_tensor(out=expm[:], in0=sc_cl[:], in1=maskA[:],
                                    op=OP.mult)
            sums_ps = psSmall.tile([1, 2 * BL], F32, tag="ps_small")
            nc.tensor.matmul(sums_ps[:], ones1f[:], expm[:], start=True, stop=True)
            nc.vector.tensor_copy(sums_sb[:], sums_ps[:])
            nc.vector.tensor_tensor(out=den[:], in0=sums_sb[0:1, 0:2 * BL:2],
                                    in1=sums_sb[0:1, 1:2 * BL:2], op=OP.add)
            nc.vector.reciprocal(rec[:], den[:])
            nc.vector.tensor_copy(
                rec2[:].rearrange("p (b c) -> p b c", b=BL),
                rec[0:1, :, None].to_broadcast([1, BL, 2]))
            rec_ps = psSmall.tile([P, 2 * BL], F32, tag="ps_small")
            nc.tensor.matmul(rec_ps[:], onesRowF[:], rec2[:], start=True, stop=True)
            nc.vector.tensor_tensor(out=alpha_f[:], in0=expm[:], in1=rec_ps[:],
                                    op=OP.mult)

            # alphas out (two transposes then clean DMAs)
            alT_sb = work.tile([BL, 2, P], F32, tag="alT_sb")
            for rc in range(2):
                alT_ps = psSmall.tile([BL, P], F32, tag="ps_small")
                nc.tensor.transpose(alT_ps[:], alpha_f[:, rc:2 * BL:2], ident_f[:])
                nc.vector.tensor_copy(alT_sb[:, rc, :], alT_ps[:])
            nc.sync.dma_start(alphas_d[:, t, 0:P], alT_sb[:, 0, :])
            nc.sync.dma_start(alphas_d[:, t, P:R], alT_sb[:, 1, 0:R - P])

            # alpha -> block-diagonal A
            nc.vector.tensor_copy(alpha_bf[:], alpha_f[:])
            for (b, rc, s0, c0, p0, n1) in scat:
                nc.sync.dma_start(
                    A_sb[p0:p0 + n1, c0, b:b + 1],
                    alpha_bf[s0:s0 + n1, 2 * b + rc:2 * b + rc + 1])

            # context = alpha-weighted sum of encoder; produce transposed ctxT
            ctxT = psUnit.tile([P, ECH, BL], BF16, tag="ps_unit")
            for q in range(4):
                ctx_ps = psA.tile([BL, VS], F32, tag="bankA")
                for c in range(NCH):
                    nc.tensor.matmul(ctx_ps[:], A_sb[:, c, :],
                                     enc_sb[:, c, q * VS:(q + 1) * VS],
                                     start=(c == 0), stop=(c == NCH - 1))
                ctx_bf = work.tile([BL, VS], BF16, tag="ctx_bf")
                nc.scalar.copy(ctx_bf[:], ctx_ps[:])
                for e4 in range(VS // P):
                    ec = q * (VS // P) + e4
                    nc.tensor.transpose(ctxT[:, ec, :],
                                        ctx_bf[:, e4 * P:(e4 + 1) * P],
                                        ident_bf[:BL, :BL])

            # gate = sigmoid(h @ W_fbeta.T + b_fbeta), fused with context
            gpre_ps = psUnit.tile([P, ECH, BL], F32, tag="ps_unit")
            for ec in range(ECH):
                wh_t = wstream.tile([P, ECH, P], BF16, tag="wst")
                nc.sync.dma_start(wh_t[:, :DCH, :],
                                  wHcat_d.ap()[ACH + ec].rearrange("c p o -> p c o"))
                for dc in range(DCH):
                    nc.tensor.matmul(gpre_ps[:, ec, :], wh_t[:, dc, :],
                                     h_rhs[:, dc, :],
                                     start=(dc == 0), stop=(dc == DCH - 1))
            nc.vector.tensor_tensor(
                out=g_sb[:], in0=gpre_ps[:],
                in1=bFbeta[:, :, None].to_broadcast([P, ECH, BL]), op=OP.add)
            nc.scalar.activation(g_sb[:], g_sb[:], AF.Sigmoid)
            nc.vector.tensor_tensor(out=gcT[:], in0=g_sb[:], in1=ctxT[:], op=OP.mult)

            # gates = h @ W_hh.T + gc @ W_ic.T   (E_proj added after)
            gate_ps = psUnit.tile([P, GCH, BL], F32, tag="ps_unit")
            for gc in range(GCH):
                wh_t = wstream.tile([P, ECH, P], BF16, tag="wst")
                nc.sync.dma_start(
                    wh_t[:, :DCH, :],
                    wHcat_d.ap()[ACH + ECH + gc].rearrange("c p o -> p c o"))
                for dc in range(DCH):
                    nc.tensor.matmul(gate_ps[:, gc, :], wh_t[:, dc, :],
                                     h_rhs[:, dc, :], start=(dc == 0), stop=False)
                wic_t = wstream.tile([P, ECH, P], BF16, tag="wst")
                nc.sync.dma_start(wic_t[:], wIc_d.ap()[gc].rearrange("c p o -> p c o"))
                for ec in range(ECH):
                    nc.tensor.matmul(gate_ps[:, gc, :], wic_t[:, ec, :],
                                     gcT[:, ec, :], start=False,
                                     stop=(ec == ECH - 1))

            nc.vector.tensor_tensor(out=gsum[:], in0=gate_ps[:],
                                    in1=eproj[:, :, t, :], op=OP.add)
            # lstm cell (gate order: i, f, g, o)
            nc.scalar.activation(act4[:, 0:DCH], gsum[:, 0:DCH], AF.Sigmoid)
            nc.scalar.activation(act4[:, DCH:2 * DCH], gsum[:, DCH:2 * DCH],
                                 AF.Sigmoid)
            nc.scalar.activation(act4[:, 2 * DCH:3 * DCH], gsum[:, 2 * DCH:3 * DCH],
                                 AF.Tanh)
            nc.scalar.activation(act4[:, 3 * DCH:4 * DCH], gsum[:, 3 * DCH:4 * DCH],
                                 AF.Sigmoid)
            c_prev, c_next = c_st[t % 2], c_st[(t + 1) % 2]
            nc.vector.tensor_tensor(out=itg[:], in0=act4[:, 0:DCH],
                                    in1=act4[:, 2 * DCH:3 * DCH], op=OP.mult)
            nc.vector.tensor_tensor(out=c_next[:], in0=act4[:, DCH:2 * DCH],
                                    in1=c_prev[:], op=OP.mult)
            nc.vector.tensor_tensor(out=c_next[:], in0=c_next[:], in1=itg[:],
                                    op=OP.add)
            nc.scalar.activation(tanh_c[:], c_next[:], AF.Tanh)
            nc.vector.tensor_tensor(out=Hbf[:, :, :, t + 1],
                                    in0=act4[:, 3 * DCH:4 * DCH], in1=tanh_c[:],
                                    op=OP.mult)

        # ---- phase C: predictions = H @ W_fc.T ----
        Hflat = const.tile([P, DCH, BL * t_steps], BF16, tag="Hflat")
        for dc in range(DCH):
            nc.vector.tensor_copy(
                Hflat[:, dc, :].rearrange("p (b t) -> p b t", b=BL),
                Hbf[:, dc, :, 1:])
        bt_chunks = []
        c0 = 0
        while c0 < BL * t_steps:
            nb = min(P, BL * t_steps - c0)
            bt_chunks.append((c0, nb))
            c0 += nb
        mmax = P
        for vs in range(NVS):
            wfc_t = wstream.tile([P, ECH, P], BF16, tag="wst")
            wfc_v = wfc_t[:].rearrange("p c o -> p (c o)")[:, :DCH * VS]
            wfc_v = wfc_v.rearrange("p (c o) -> p c o", c=DCH)
            nc.sync.dma_start(wfc_v, wFc_d.ap()[vs].rearrange("c p o -> p c o"))
            vcols = min(VS, V - vs * VS)
            for (c0, m) in bt_chunks:
                p3 = psA.tile([mmax, VS], F32, tag="bankA")
                for dc in range(DCH):
                    nc.tensor.matmul(p3[:m, :], Hflat[:, dc, c0:c0 + m],
                                     wfc_v[:, dc, :],
                                     start=(dc == 0),
                                     stop=(dc == DCH - 1 and not need_bfc))
                if need_bfc:
                    nc.tensor.matmul(p3[:m, :], onesRow[:, :m],
                                     bFc[:, vs * VS:(vs + 1) * VS],
                                     start=False, stop=True)
                pred_sb = work.tile([mmax, VS], F32, tag="pred_sb", bufs=2)
                nc.any.tensor_copy(pred_sb[:m, :], p3[:m, :])
                nc.sync.dma_start(
                    preds_d[c0:c0 + m, vs * VS:vs * VS + vcols],
                    pred_sb[:m, :vcols])

    nc.compile()
    return nc


def _host_prep(inputs, t_steps=T):
    """Shard + lay out inputs for the 8 per-core programs."""
    nt = BL * t_steps
    ntp = ((nt + P - 1) // P) * P
    enc = np.asarray(inputs["encoder_out"], np.float32)
    cap = np.asarray(inputs["captions"]).astype(np.int32)

    wE = _tiles(_bf(np.asarray(inputs["W_enc_att"], np.float32).T), ECH, ACH)
    bEnc = np.ascontiguousarray(
        np.asarray(inputs["b_enc_att"], np.float32).reshape(ACH, P).T)
    bDec = np.ascontiguousarray(
        np.asarray(inputs["b_dec_att"], np.float32).reshape(ACH, P).T)
    whcat = np.concatenate([
        np.asarray(inputs["W_dec_att"], np.float32).T,
        np.asarray(inputs["W_fbeta"], np.float32).T,
        np.asarray(inputs["W_hh"], np.float32).T], axis=1)
    wHcat = _tiles(_bf(whcat), DCH, ACH + ECH + GCH)
    w_ih = np.asarray(inputs["W_ih"], np.float32)
    wIc = _tiles(_bf(w_ih[:, EMB:].T), ECH, GCH)
    wIe = _tiles(_bf(w_ih[:, :EMB].T), MCH, GCH)
    biasG = np.ascontiguousarray(
        (np.asarray(inputs["b_ih"], np.float32)
         + np.asarray(inputs["b_hh"], np.float32)).reshape(GCH, P).T)
    bFbeta = np.ascontiguousarray(
        np.asarray(inputs["b_fbeta"], np.float32).reshape(ECH, P).T)
    wFull = np.ascontiguousarray(
        _bf(np.asarray(inputs["W_full_att"], np.float32)[0].reshape(ACH, P).T))
    wInitH = _tiles(_bf(np.asarray(inputs["W_init_h"], np.float32).T), ECH, DCH)
    wInitC = _tiles(_bf(np.asarray(inputs["W_init_c"], np.float32).T), ECH, DCH)
    bInitH = np.ascontiguousarray(
        np.asarray(inputs["b_init_h"], np.float32).reshape(DCH, P).T)
    bInitC = np.ascontiguousarray(
        np.asarray(inputs["b_init_c"], np.float32).reshape(DCH, P).T)
    maskA = np.zeros((P, 2 * BL), np.float32)
    maskA[:, 0::2] = 1.0
    maskA[:R - P, 1::2] = 1.0
    onesBlk = np.zeros((BRP, BL), np.float32)
    for b in range(BL):
        onesBlk[b * R:(b + 1) * R, b] = 1.0 / R
    onesBlk = _bf(onesBlk)
    wfcT = np.zeros((D, VP), np.float32)
    wfcT[:, :V] = np.asarray(inputs["W_fc"], np.float32).T
    wFc = np.ascontiguousarray(
        _bf(wfcT).reshape(DCH, P, NVS, VS).transpose(2, 0, 1, 3))
    bFc = np.zeros((1, VP), np.float32)
    bFc[0, :V] = np.asarray(inputs["b_fc"], np.float32)
    need_bfc = bool(np.any(bFc != 0))
    bFc = _bf(bFc)
    emb_tab = np.asarray(inputs["emb_table"], np.float32)

    shared = dict(emb=emb_tab, wE=wE, bEnc=bEnc, bDec=bDec, wHcat=wHcat, wIc=wIc,
                  wIe=wIe, biasG=biasG, bFbeta=bFbeta, wFull=wFull, wInitH=wInitH,
                  wInitC=wInitC, bInitH=bInitH, bInitC=bInitC, maskA=maskA,
                  onesBlk=onesBlk, wFc=wFc, bFc=bFc)

    in_maps = []
    for k in range(NCORES):
        bsl = slice(k * BL, (k + 1) * BL)
        e = enc[bsl]                                   # [BL, R, E]
        e_flat = np.zeros((BRP, E), np.float32)
        e_flat[:BR] = e.reshape(BR, E)
        encT = np.ascontiguousarray(e.reshape(BR, E).T)  # [E, BR]
        cap_ids = np.zeros((ntp,), np.int32)
        cap_ids[:nt] = cap[bsl, :t_steps].reshape(-1)
        m = dict(shared)
        m["enc"] = _bf(e_flat)
        m["encT"] = _bf(encT)
        m["cap"] = cap_ids
        in_maps.append(m)
    return in_maps, need_bfc


def kernel(**inputs):
    t_steps = np.asarray(inputs["captions"]).shape[1] - 1
    in_maps, need_bfc = _host_prep(inputs, t_steps)
    key = (t_steps, need_bfc)
    if key not in _CACHE:
        _CACHE[key] = build_nc(t_steps, need_bfc)
    nc = _CACHE[key]
    res = run_bass_kernel_spmd(nc, in_maps, core_ids=list(range(NCORES)))
    preds = np.concatenate(
        [r["preds"].reshape(BL, t_steps, V) for r in res.results], axis=0)
    alphas = np.concatenate([r["alphas"] for r in res.results], axis=0)
    return preds, alphas
